# revision 22
# baseline (speedup 1.0000x reference)
"""Trainium2 Bass kernel for MultiHeadLatentAttention (B=2, S=2048, H=2048,
NH=16, HD=128, LAT=512), SPMD across 8 NeuronCores.

Sharding: 8 cores = 2 (batch) x 4 (head-group TP). Core c handles batch c//4
and head group j = c%4 = heads {j, 4+j, 8+j, 12+j}. That grouping is chosen so
the 4 heads share exactly 256 rows of Wq_up/Wk_up: heads j and 4+j are the raw
x1/x2 slices of q_half, heads 8+j and 12+j are their RoPE combinations — so
the up-projection shards 4-way with no duplication. Each core computes its
partial o_proj output; the host sums the 4 partials per batch and adds bo.

Self-contained: builds + compiles the Bass program on first call (cached),
runs via run_bass_kernel_spmd on cores 0-7.
"""
import os
import sys
import types
from contextlib import ExitStack

import numpy as np

if "/opt/trn_rl_repo" not in sys.path:
    sys.path.insert(0, "/opt/trn_rl_repo")

import ml_dtypes

# ---------------------------------------------------------------------------
# NTFF-profile shim: antenv.axon_hooks is missing in this image; register a
# hook backed by the axon PJRT .so so trace=True can capture HW exec time.
# ---------------------------------------------------------------------------


def _install_axon_hooks_shim():
    if "antenv.axon_hooks" in sys.modules:
        return
    try:
        import antenv
        from trn_agent_boot.trn_boot import _ntff_profile_via_ctypes
        hook = _ntff_profile_via_ctypes("/opt/axon/libaxon_pjrt.so")
    except Exception:
        return
    mod = types.ModuleType("antenv.axon_hooks")
    mod.get_axon_ntff_profile_hook = lambda: hook
    mod.set_axon_ntff_profile_hook = lambda h: None
    sys.modules["antenv.axon_hooks"] = mod
    antenv.axon_hooks = mod


_install_axon_hooks_shim()

import concourse.bass as bass  # noqa: E402
import concourse.mybir as mybir  # noqa: E402
import concourse.tile as tile  # noqa: E402
from concourse import bacc  # noqa: E402
from concourse.bass_utils import run_bass_kernel_spmd  # noqa: E402

P = 128
H = 2048
NH = 16
HD = 128
LAT = 512
B = 2
S = 2048
ROPE_DIM = H // 4
NHG = 4          # heads per core
SC = 512         # s/q chunk (one PSUM bank of fp32)
INV_SQRT_HD = 0.08838834764831845  # 1/sqrt(128)

f32 = mybir.dt.float32
f32r = mybir.dt.float32r
bf16 = mybir.dt.bfloat16
f16 = mybir.dt.float16
Act = mybir.ActivationFunctionType
Alu = mybir.AluOpType
BF16 = ml_dtypes.bfloat16
F16 = np.float16


def build_mla(seq=S, debug=False):
    """Build one core's program. All cores run this same program SPMD."""
    NSC = seq // SC   # s-chunks
    HT = H // P       # 16 h-tiles
    LT = LAT // P     # 4 l-tiles
    ST = seq // P     # s-tiles (= k-tiles in attention)

    nc = bacc.Bacc("TRN2", target_bir_lowering=False, debug=debug)

    hsT = nc.dram_tensor("hsT", [H, seq], f16, kind="ExternalInput")
    WqdT = nc.dram_tensor("WqdT", [H, LAT], f16, kind="ExternalInput")
    WkvdT = nc.dram_tensor("WkvdT", [H, LAT], f16, kind="ExternalInput")
    bqd = nc.dram_tensor("bqd", [LAT], f32, kind="ExternalInput")
    bkvd = nc.dram_tensor("bkvd", [LAT], f32, kind="ExternalInput")
    WquT = nc.dram_tensor("WquT", [LAT, 2 * P], f32r, kind="ExternalInput")
    WkuT = nc.dram_tensor("WkuT", [LAT, 2 * P], f32r, kind="ExternalInput")
    bqku = nc.dram_tensor("bqku", [P, 4], f32, kind="ExternalInput")
    WvuT = nc.dram_tensor("WvuT", [LAT, NHG * P], f32r, kind="ExternalInput")
    bvu = nc.dram_tensor("bvu", [1, NHG * P], f32, kind="ExternalInput")
    WoT = nc.dram_tensor("WoT", [NHG * P, H], f16, kind="ExternalInput")
    cosT = nc.dram_tensor("cosT", [P, seq], f16, kind="ExternalInput")
    sinT = nc.dram_tensor("sinT", [P, seq], f16, kind="ExternalInput")
    ones = nc.dram_tensor("ones", [P, P], f16, kind="ExternalInput")
    outT = nc.dram_tensor("outT", [H, seq], f16, kind="ExternalOutput")

    def r(ap):  # fast fp32 matmul path
        return ap.bitcast(f32r)

    with tile.TileContext(nc) as tc, ExitStack() as top:
        const = top.enter_context(tc.tile_pool(name="const", bufs=1))
        ao_pool = top.enter_context(tc.tile_pool(name="ao", bufs=1))

        bqd_t = const.tile([P, LT], f32)
        nc.sync.dma_start(bqd_t[:], bqd.rearrange("(o p) -> p o", p=P))
        bkvd_t = const.tile([P, LT], f32)
        nc.sync.dma_start(bkvd_t[:], bkvd.rearrange("(o p) -> p o", p=P))
        bqku_t = const.tile([P, 4], f32)
        nc.sync.dma_start(bqku_t[:], bqku[:])
        bvu_bc = const.tile([P, NHG * P], f32)
        nc.sync.dma_start(bvu_bc[:], bvu[:].to_broadcast((P, NHG * P)))
        ones_r = const.tile([P, P], f16)
        nc.sync.dma_start(ones_r[:], ones[:])

        # HAM warmup: ~64 back-to-back matmuls (~3.5us of PE activity) while
        # the initial weight/activation DMAs stream in, so the first real
        # matmuls run at 2.4GHz instead of the cold 1.2GHz.
        with tc.tile_pool(name="warm", bufs=1, space="PSUM") as warm_pool:
            wps = warm_pool.tile([P, P], f32)
            for _ in range(64):
                nc.tensor.matmul(wps[:], ones_r[:], ones_r[:],
                                 start=True, stop=True)

        attn_outT = ao_pool.tile([P, NHG, seq], f16)

        with ExitStack() as qkv_scope:
            qk_pool = qkv_scope.enter_context(tc.tile_pool(name="qk", bufs=1))
            v_pool = qkv_scope.enter_context(tc.tile_pool(name="v", bufs=1))
            qT = qk_pool.tile([P, NHG, seq], f16)  # 0=x1, 1=x2, 2,3=rope
            kT = qk_pool.tile([P, NHG, seq], f16)
            v_bf = v_pool.tile([P, ST, NHG * P], f16)  # token-major v

            with ExitStack() as lat_scope:
                lat_pool = lat_scope.enter_context(
                    tc.tile_pool(name="lat", bufs=1))
                q_latT = lat_pool.tile([P, LT, seq], f32r)
                kv_latT = lat_pool.tile([P, LT, seq], f32r)

                # ---------------- phase D: down projections ----------------
                with tc.tile_pool(name="wd", bufs=1) as wd_pool, \
                     tc.tile_pool(name="hst", bufs=HT + 4) as hst_pool, \
                     tc.tile_pool(name="psd", bufs=4, space="PSUM") as psd:
                    # per-ht DMA split, interleaved with the first s-chunk's
                    # activation tiles, so matmul (m=0, ht=0) can start after
                    # ~400KB of DMA instead of the full 12MB initial load
                    wqd_t = wd_pool.tile([P, HT, LAT], f16)
                    wkvd_t = wd_pool.tile([P, HT, LAT], f16)
                    hts0 = []
                    for ht in range(HT):
                        t = hst_pool.tile([P, SC], f16, tag="hst")
                        nc.sync.dma_start(t[:], hsT[ht * P:(ht + 1) * P, :SC])
                        hts0.append(t)
                        nc.sync.dma_start(
                            wqd_t[:, ht, :], WqdT[ht * P:(ht + 1) * P, :])
                        nc.sync.dma_start(
                            wkvd_t[:, ht, :], WkvdT[ht * P:(ht + 1) * P, :])

                    for sc in range(NSC):
                        ssl = slice(sc * SC, (sc + 1) * SC)
                        if sc == 0:
                            hts = hts0
                        else:
                            hts = []
                            for ht in range(HT):
                                t = hst_pool.tile([P, SC], f16, tag="hst")
                                nc.sync.dma_start(
                                    t[:], hsT[ht * P:(ht + 1) * P, ssl])
                                hts.append(t)
                        for m in range(2 * LT):
                            w_t, dst, b_t = ((wqd_t, q_latT, bqd_t) if m < LT
                                             else (wkvd_t, kv_latT, bkvd_t))
                            lt = m % LT
                            ps = psd.tile([P, SC], f32)
                            for ht in range(HT):
                                nc.tensor.matmul(
                                    ps[:],
                                    w_t[:, ht, lt * P:(lt + 1) * P],
                                    hts[ht][:],
                                    start=(ht == 0), stop=(ht == HT - 1))
                            nc.scalar.activation(
                                dst[:, lt, ssl], ps[:], Act.Identity,
                                bias=b_t[:, lt:lt + 1])

                # ------------- phase U: up projections + rope --------------
                with tc.tile_pool(name="wu", bufs=1) as wu_pool, \
                     tc.tile_pool(name="psu", bufs=4, space="PSUM") as psu, \
                     tc.tile_pool(name="ut", bufs=4) as ut_pool:
                    cos_t = wu_pool.tile([P, seq], f16)
                    nc.sync.dma_start(cos_t[:], cosT[:])
                    sin_t = wu_pool.tile([P, seq], f16)
                    nc.sync.dma_start(sin_t[:], sinT[:])
                    wqu_t = wu_pool.tile([P, LT, 2 * P], f32r)
                    nc.sync.dma_start(
                        wqu_t[:], WquT.rearrange("(lt p) m -> p lt m", p=P))
                    wku_t = wu_pool.tile([P, LT, 2 * P], f32r)
                    nc.sync.dma_start(
                        wku_t[:], WkuT.rearrange("(lt p) m -> p lt m", p=P))
                    wvu_t = wu_pool.tile([P, LT, NHG * P], f32r)
                    nc.sync.dma_start(
                        wvu_t[:], WvuT.rearrange("(lt p) m -> p lt m", p=P))

                    for sc in range(NSC):
                        ssl = slice(sc * SC, (sc + 1) * SC)
                        for ci in range(4):  # q_x1, q_x2, k_x1, k_x2
                            w_t = wqu_t if ci < 2 else wku_t
                            latsrc = q_latT if ci < 2 else kv_latT
                            csl = slice((ci % 2) * P, (ci % 2) * P + P)
                            dstT = qT if ci < 2 else kT
                            ps = psu.tile([P, SC], f32, tag="psu_qk")
                            for lt in range(LT):
                                nc.tensor.matmul(
                                    ps[:], w_t[:, lt, csl],
                                    latsrc[:, lt, ssl],
                                    start=(lt == 0), stop=(lt == LT - 1))
                            # bias-add on DVE (free-dim broadcast of [P,1])
                            # to keep ACT free for the attention exps
                            nc.vector.tensor_tensor(
                                dstT[:, ci % 2, ssl], ps[:],
                                bqku_t[:, ci:ci + 1].to_broadcast((P, SC)),
                                Alu.add)
                        # rope: slot2 = x1*cos - x2*sin, slot3 = x1*sin + x2*cos
                        for dstT in (qT, kT):
                            x1 = dstT[:, 0, ssl]
                            x2 = dstT[:, 1, ssl]
                            t1 = ut_pool.tile([P, SC], f16, tag="ropetmp")
                            t2 = ut_pool.tile([P, SC], f16, tag="ropetmp")
                            nc.vector.tensor_mul(t1[:], x1, cos_t[:, ssl])
                            nc.vector.tensor_mul(t2[:], x2, sin_t[:, ssl])
                            nc.vector.tensor_sub(dstT[:, 2, ssl], t1[:], t2[:])
                            t3 = ut_pool.tile([P, SC], f16, tag="ropetmp")
                            t4 = ut_pool.tile([P, SC], f16, tag="ropetmp")
                            nc.vector.tensor_mul(t3[:], x1, sin_t[:, ssl])
                            nc.vector.tensor_mul(t4[:], x2, cos_t[:, ssl])
                            nc.vector.tensor_add(dstT[:, 3, ssl], t3[:], t4[:])

                    # v: token-major, kv_lat as stationary operand
                    for st in range(ST):
                        ps = psu.tile([P, NHG * P], f32, tag="psu_v")
                        for lt in range(LT):
                            nc.tensor.matmul(
                                ps[:], kv_latT[:, lt, st * P:(st + 1) * P],
                                wvu_t[:, lt, :],
                                start=(lt == 0), stop=(lt == LT - 1))
                        nc.vector.tensor_tensor(
                            v_bf[:, st, :], ps[:], bvu_bc[:], Alu.add)

            # ------- phase A+O: attention with o_proj interleaved --------
            # qc-outer: once all 4 heads of a q-chunk are normalized, that
            # chunk's o_proj runs on PE underneath the next chunk's
            # ACT-bound score/exp pipeline.
            with tc.tile_pool(name="exp", bufs=2) as exp_pool, \
                 tc.tile_pool(name="wo", bufs=1) as wo_pool, \
                 tc.tile_pool(name="pss", bufs=2, space="PSUM") as pss, \
                 tc.tile_pool(name="psav", bufs=1, space="PSUM") as psav, \
                 tc.tile_pool(name="pssm", bufs=1, space="PSUM") as pssm, \
                 tc.tile_pool(name="pso", bufs=2, space="PSUM") as pso, \
                 tc.tile_pool(name="att", bufs=3) as at_pool, \
                 tc.tile_pool(name="ot", bufs=4) as ot_pool:
                # o_proj weights resident: [c-part, ct, m] layout
                wo_t = wo_pool.tile([P, NHG, H], f16)
                for ct in range(NHG):
                    nc.sync.dma_start(
                        wo_t[:, ct, :], WoT[ct * P:(ct + 1) * P, :])

                for qc in range(NSC):
                    qsl = slice(qc * SC, (qc + 1) * SC)
                    for h in range(NHG):
                        expt = exp_pool.tile([P, ST, SC], f16, tag="expt")
                        # k-tiles in pairs: two matmuls fill a 2-bank psum
                        # tile, one ACT exp covers both (amortizes the ~240ns
                        # per-ACT-instruction overhead)
                        for kth in range(ST // 2):
                            ps = pss.tile([P, 2, SC], f32, tag="score")
                            for half in (0, 1):
                                kt = 2 * kth + half
                                nc.tensor.matmul(
                                    ps[:, half, :],
                                    kT[:, h, kt * P:(kt + 1) * P],
                                    qT[:, h, qsl], start=True, stop=True)
                            nc.scalar.activation(
                                expt[:, 2 * kth:2 * kth + 2, :], ps[:],
                                Act.Exp, scale=INV_SQRT_HD)
                        # AV accumulation on PE
                        pav = psav.tile([P, SC], f32, tag="av")
                        for kt in range(ST):
                            nc.tensor.matmul(
                                pav[:], v_bf[:, kt, h * P:(h + 1) * P],
                                expt[:, kt, :],
                                start=(kt == 0), stop=(kt == ST - 1))
                        # softmax denominators: DVE-accumulate exp tiles,
                        # then one ones-matmul = 128-way partition reduce
                        # broadcast to all partitions.
                        acc = at_pool.tile([P, SC], f16, tag="acc")
                        nc.vector.tensor_add(
                            acc[:], expt[:, 0, :], expt[:, 1, :])
                        for kt in range(2, ST):
                            nc.vector.tensor_add(
                                acc[:], acc[:], expt[:, kt, :])
                        psm = pssm.tile([P, SC], f32, tag="sum")
                        nc.tensor.matmul(
                            psm[:], ones_r[:], acc[:], start=True, stop=True)
                        rec_bc = at_pool.tile([P, SC], f32, tag="rec_bc")
                        nc.vector.reciprocal_approx_fast(rec_bc[:], psm[:])
                        nc.vector.tensor_tensor(
                            attn_outT[:, h, qsl], pav[:], rec_bc[:], Alu.mult)
                    # o_proj for this q-chunk
                    for mt in range(H // P):
                        msl = slice(mt * P, (mt + 1) * P)
                        ps = pso.tile([P, SC], f32, tag="opsum")
                        for ct in range(NHG):
                            nc.tensor.matmul(
                                ps[:], wo_t[:, ct, msl],
                                attn_outT[:, ct, qsl],
                                start=(ct == 0), stop=(ct == NHG - 1))
                        ot = ot_pool.tile([P, SC], f16, tag="ot")
                        nc.vector.tensor_copy(ot[:], ps[:])
                        nc.sync.dma_start(outT[msl, qsl], ot[:])

    nc.compile()
    return nc


# ---------------------------------------------------------------------------
# Host side: shard inputs, run SPMD, gather.
# ---------------------------------------------------------------------------

def _rope_cos_sin(seq_len, dim, base=10000.0):
    inv_freq = 1.0 / (base ** (np.arange(0, dim, 2, dtype=np.float32) / dim))
    t = np.arange(seq_len, dtype=np.float32)
    freqs = np.outer(t, inv_freq).astype(np.float32)
    emb = np.concatenate([freqs, freqs], -1)
    return np.cos(emb).astype(np.float32), np.sin(emb).astype(np.float32)


def make_in_maps(hidden_states, Wq_down, bq_down, Wkv_down, bkv_down,
                 Wq_up, bq_up, Wk_up, bk_up, Wv_up, bv_up, Wo, bo):
    cos, sin = _rope_cos_sin(S, ROPE_DIM)
    WqdT = np.ascontiguousarray(Wq_down.T).astype(F16)
    WkvdT = np.ascontiguousarray(Wkv_down.T).astype(F16)
    hsT = [np.ascontiguousarray(hidden_states[b].T).astype(F16)
           for b in range(B)]
    in_maps = []
    for c in range(8):
        b, j = c // 4, c % 4
        heads = [j, 4 + j, 8 + j, 12 + j]
        x1 = slice(j * P, (j + 1) * P)
        x2 = slice(512 + j * P, 512 + (j + 1) * P)
        vrows = np.concatenate(
            [np.arange(h * P, (h + 1) * P) for h in heads])
        in_maps.append(dict(
            hsT=hsT[b],
            WqdT=WqdT, WkvdT=WkvdT,
            bqd=np.ascontiguousarray(bq_down),
            bkvd=np.ascontiguousarray(bkv_down),
            WquT=np.ascontiguousarray(
                np.concatenate([Wq_up[x1], Wq_up[x2]], 0).T),
            WkuT=np.ascontiguousarray(
                np.concatenate([Wk_up[x1], Wk_up[x2]], 0).T),
            bqku=np.stack(
                [bq_up[x1], bq_up[x2], bk_up[x1], bk_up[x2]], axis=1).copy(),
            WvuT=np.ascontiguousarray(Wv_up[vrows].T),
            bvu=np.ascontiguousarray(bv_up[vrows][None, :]),
            WoT=np.ascontiguousarray(Wo[:, vrows].T).astype(F16),
            cosT=np.ascontiguousarray(cos[:, x1].T).astype(F16),
            sinT=np.ascontiguousarray(sin[:, x1].T).astype(F16),
            ones=np.ones((P, P), np.float16),
        ))
    return in_maps


_NC_CACHE = {}


def _get_nc():
    if "nc" not in _NC_CACHE:
        _NC_CACHE["nc"] = build_mla()
    return _NC_CACHE["nc"]


LAST_RESULTS = None  # BassKernelResults of the most recent kernel() call


def kernel(**inputs):
    global LAST_RESULTS
    nc = _get_nc()
    in_maps = make_in_maps(**inputs)
    trace = bool(int(os.environ.get("MLA_TRACE", "0")))
    kwargs = {}
    if trace:
        tc_env = os.environ.get("MLA_TRACE_CORES", "0,1,2,3,4,5,6,7")
        kwargs["trace_cores"] = [int(x) for x in tc_env.split(",")]
    res = run_bass_kernel_spmd(
        nc, in_maps, core_ids=list(range(8)), trace=trace, **kwargs)
    LAST_RESULTS = res
    bo = inputs["bo"]
    out = np.zeros((B, S, H), np.float32)
    for b in range(B):
        acc = res.results[b * 4]["outT"].astype(np.float32)
        for j in range(1, 4):
            acc = acc + res.results[b * 4 + j]["outT"]
        out[b] = acc.T + bo[None, :]
    return out


# revision 26
# speedup vs baseline: 1.0147x; 1.0147x over previous
"""Trainium2 Bass kernel for MultiHeadLatentAttention (B=2, S=2048, H=2048,
NH=16, HD=128, LAT=512), SPMD across 8 NeuronCores.

Sharding: 8 cores = 2 (batch) x 4 (head-group TP). Core c handles batch c//4
and head group j = c%4 = heads {j, 4+j, 8+j, 12+j}. That grouping is chosen so
the 4 heads share exactly 256 rows of Wq_up/Wk_up: heads j and 4+j are the raw
x1/x2 slices of q_half, heads 8+j and 12+j are their RoPE combinations — so
the up-projection shards 4-way with no duplication. Each core computes its
partial o_proj output; the host sums the 4 partials per batch and adds bo.

Self-contained: builds + compiles the Bass program on first call (cached),
runs via run_bass_kernel_spmd on cores 0-7.
"""
import os
import sys
import types
from contextlib import ExitStack

import numpy as np

if "/opt/trn_rl_repo" not in sys.path:
    sys.path.insert(0, "/opt/trn_rl_repo")

import ml_dtypes

# ---------------------------------------------------------------------------
# NTFF-profile shim: antenv.axon_hooks is missing in this image; register a
# hook backed by the axon PJRT .so so trace=True can capture HW exec time.
# ---------------------------------------------------------------------------


def _install_axon_hooks_shim():
    if "antenv.axon_hooks" in sys.modules:
        return
    try:
        import antenv
        from trn_agent_boot.trn_boot import _ntff_profile_via_ctypes
        hook = _ntff_profile_via_ctypes("/opt/axon/libaxon_pjrt.so")
    except Exception:
        return
    mod = types.ModuleType("antenv.axon_hooks")
    mod.get_axon_ntff_profile_hook = lambda: hook
    mod.set_axon_ntff_profile_hook = lambda h: None
    sys.modules["antenv.axon_hooks"] = mod
    antenv.axon_hooks = mod


_install_axon_hooks_shim()

import concourse.bass as bass  # noqa: E402
import concourse.mybir as mybir  # noqa: E402
import concourse.tile as tile  # noqa: E402
from concourse import bacc  # noqa: E402
from concourse.bass_utils import run_bass_kernel_spmd  # noqa: E402

P = 128
H = 2048
NH = 16
HD = 128
LAT = 512
B = 2
S = 2048
ROPE_DIM = H // 4
NHG = 4          # heads per core
SC = 512         # s/q chunk (one PSUM bank of fp32)
INV_SQRT_HD = 0.08838834764831845  # 1/sqrt(128)

f32 = mybir.dt.float32
f32r = mybir.dt.float32r
bf16 = mybir.dt.bfloat16
f16 = mybir.dt.float16
Act = mybir.ActivationFunctionType
Alu = mybir.AluOpType
BF16 = ml_dtypes.bfloat16
F16 = np.float16


def build_mla(seq=S, debug=False):
    """Build one core's program. All cores run this same program SPMD."""
    NSC = seq // SC   # s-chunks
    HT = H // P       # 16 h-tiles
    LT = LAT // P     # 4 l-tiles
    ST = seq // P     # s-tiles (= k-tiles in attention)

    nc = bacc.Bacc("TRN2", target_bir_lowering=False, debug=debug)

    hsT = nc.dram_tensor("hsT", [H, seq], f16, kind="ExternalInput")
    WqdT = nc.dram_tensor("WqdT", [H, LAT], f16, kind="ExternalInput")
    WkvdT = nc.dram_tensor("WkvdT", [H, LAT], f16, kind="ExternalInput")
    bqd = nc.dram_tensor("bqd", [LAT], f32, kind="ExternalInput")
    bkvd = nc.dram_tensor("bkvd", [LAT], f32, kind="ExternalInput")
    WquT = nc.dram_tensor("WquT", [LAT, 2 * P], f32r, kind="ExternalInput")
    WkuT = nc.dram_tensor("WkuT", [LAT, 2 * P], f32r, kind="ExternalInput")
    bqku = nc.dram_tensor("bqku", [P, 4], f32, kind="ExternalInput")
    WvuT = nc.dram_tensor("WvuT", [LAT, NHG * P], f32r, kind="ExternalInput")
    bvu = nc.dram_tensor("bvu", [1, NHG * P], f32, kind="ExternalInput")
    WoT = nc.dram_tensor("WoT", [NHG * P, H], f16, kind="ExternalInput")
    cosT = nc.dram_tensor("cosT", [P, seq], f16, kind="ExternalInput")
    sinT = nc.dram_tensor("sinT", [P, seq], f16, kind="ExternalInput")
    ones = nc.dram_tensor("ones", [P, P], f16, kind="ExternalInput")
    outT = nc.dram_tensor("outT", [H, seq], f16, kind="ExternalOutput")

    def r(ap):  # fast fp32 matmul path
        return ap.bitcast(f32r)

    with tile.TileContext(nc) as tc, ExitStack() as top:
        const = top.enter_context(tc.tile_pool(name="const", bufs=1))
        ao_pool = top.enter_context(tc.tile_pool(name="ao", bufs=1))

        bqd_t = const.tile([P, LT], f32)
        nc.sync.dma_start(bqd_t[:], bqd.rearrange("(o p) -> p o", p=P))
        bkvd_t = const.tile([P, LT], f32)
        nc.sync.dma_start(bkvd_t[:], bkvd.rearrange("(o p) -> p o", p=P))
        bqku_t = const.tile([P, 4], f32)
        nc.sync.dma_start(bqku_t[:], bqku[:])
        bvu_bc = const.tile([P, NHG * P], f32)
        nc.sync.dma_start(bvu_bc[:], bvu[:].to_broadcast((P, NHG * P)))
        ones_r = const.tile([P, P], f16)
        nc.sync.dma_start(ones_r[:], ones[:])

        # HAM warmup: ~64 back-to-back matmuls (~3.5us of PE activity) while
        # the initial weight/activation DMAs stream in, so the first real
        # matmuls run at 2.4GHz instead of the cold 1.2GHz.
        with tc.tile_pool(name="warm", bufs=1, space="PSUM") as warm_pool:
            wtiles = [warm_pool.tile([P, P], f32, tag=f"w{i}", name=f"warm{i}")
                      for i in range(4)]
            for i in range(48):
                nc.tensor.matmul(wtiles[i % 4][:], ones_r[:], ones_r[:],
                                 start=True, stop=True)

        attn_outT = ao_pool.tile([P, NHG, seq], f16)

        with ExitStack() as qkv_scope:
            qk_pool = qkv_scope.enter_context(tc.tile_pool(name="qk", bufs=1))
            v_pool = qkv_scope.enter_context(tc.tile_pool(name="v", bufs=1))
            qT = qk_pool.tile([P, NHG, seq], f16)  # 0=x1, 1=x2, 2,3=rope
            kT = qk_pool.tile([P, NHG, seq], f16)
            v_bf = v_pool.tile([P, ST, NHG * P], f16)  # token-major v

            with ExitStack() as lat_scope:
                lat_pool = lat_scope.enter_context(
                    tc.tile_pool(name="lat", bufs=1))
                q_latT = lat_pool.tile([P, LT, seq], f32r)
                kv_latT = lat_pool.tile([P, LT, seq], f32r)

                # ---------------- phase D: down projections ----------------
                with tc.tile_pool(name="wd", bufs=1) as wd_pool, \
                     tc.tile_pool(name="hst", bufs=HT + 4) as hst_pool, \
                     tc.tile_pool(name="psd", bufs=4, space="PSUM") as psd:
                    # per-ht DMA split, interleaved with the first s-chunk's
                    # activation tiles, so matmul (m=0, ht=0) can start after
                    # ~400KB of DMA instead of the full 12MB initial load
                    wqd_t = wd_pool.tile([P, HT, LAT], f16)
                    wkvd_t = wd_pool.tile([P, HT, LAT], f16)
                    hts0 = []
                    for ht in range(HT):
                        t = hst_pool.tile([P, SC], f16, tag="hst")
                        nc.sync.dma_start(t[:], hsT[ht * P:(ht + 1) * P, :SC])
                        hts0.append(t)
                        nc.sync.dma_start(
                            wqd_t[:, ht, :], WqdT[ht * P:(ht + 1) * P, :])
                        nc.sync.dma_start(
                            wkvd_t[:, ht, :], WkvdT[ht * P:(ht + 1) * P, :])

                    for sc in range(NSC):
                        ssl = slice(sc * SC, (sc + 1) * SC)
                        if sc == 0:
                            hts = hts0
                        else:
                            hts = []
                            for ht in range(HT):
                                t = hst_pool.tile([P, SC], f16, tag="hst")
                                nc.sync.dma_start(
                                    t[:], hsT[ht * P:(ht + 1) * P, ssl])
                                hts.append(t)
                        for m in range(2 * LT):
                            w_t, dst, b_t = ((wqd_t, q_latT, bqd_t) if m < LT
                                             else (wkvd_t, kv_latT, bkvd_t))
                            lt = m % LT
                            ps = psd.tile([P, SC], f32)
                            for ht in range(HT):
                                nc.tensor.matmul(
                                    ps[:],
                                    w_t[:, ht, lt * P:(lt + 1) * P],
                                    hts[ht][:],
                                    start=(ht == 0), stop=(ht == HT - 1))
                            nc.scalar.activation(
                                dst[:, lt, ssl], ps[:], Act.Identity,
                                bias=b_t[:, lt:lt + 1])

                # ------------- phase U: up projections + rope --------------
                with tc.tile_pool(name="wu", bufs=1) as wu_pool, \
                     tc.tile_pool(name="psu", bufs=4, space="PSUM") as psu, \
                     tc.tile_pool(name="ut", bufs=4) as ut_pool:
                    cos_t = wu_pool.tile([P, seq], f16)
                    nc.sync.dma_start(cos_t[:], cosT[:])
                    sin_t = wu_pool.tile([P, seq], f16)
                    nc.sync.dma_start(sin_t[:], sinT[:])
                    wqu_t = wu_pool.tile([P, LT, 2 * P], f32r)
                    nc.sync.dma_start(
                        wqu_t[:], WquT.rearrange("(lt p) m -> p lt m", p=P))
                    wku_t = wu_pool.tile([P, LT, 2 * P], f32r)
                    nc.sync.dma_start(
                        wku_t[:], WkuT.rearrange("(lt p) m -> p lt m", p=P))
                    wvu_t = wu_pool.tile([P, LT, NHG * P], f32r)
                    nc.sync.dma_start(
                        wvu_t[:], WvuT.rearrange("(lt p) m -> p lt m", p=P))

                    # v first (every AV chain in phase A needs ALL v tiles),
                    # then k_x1/q_x1 before x2 so attention head 0 can start
                    # while the rest of phase U still runs.
                    for st in range(ST):
                        ps = psu.tile([P, NHG * P], f32, tag="psu_v")
                        for lt in range(LT):
                            nc.tensor.matmul(
                                ps[:], kv_latT[:, lt, st * P:(st + 1) * P],
                                wvu_t[:, lt, :],
                                start=(lt == 0), stop=(lt == LT - 1))
                        nc.vector.tensor_tensor(
                            v_bf[:, st, :], ps[:], bvu_bc[:], Alu.add)

                    for ci in (2, 0, 3, 1):  # k_x1, q_x1, k_x2, q_x2
                        w_t = wqu_t if ci < 2 else wku_t
                        latsrc = q_latT if ci < 2 else kv_latT
                        csl = slice((ci % 2) * P, (ci % 2) * P + P)
                        dstT = qT if ci < 2 else kT
                        for sc in range(NSC):
                            ssl = slice(sc * SC, (sc + 1) * SC)
                            ps = psu.tile([P, SC], f32, tag="psu_qk")
                            for lt in range(LT):
                                nc.tensor.matmul(
                                    ps[:], w_t[:, lt, csl],
                                    latsrc[:, lt, ssl],
                                    start=(lt == 0), stop=(lt == LT - 1))
                            # bias-add on DVE (free-dim broadcast of [P,1])
                            # to keep ACT free for the attention exps
                            nc.vector.tensor_tensor(
                                dstT[:, ci % 2, ssl], ps[:],
                                bqku_t[:, ci:ci + 1].to_broadcast((P, SC)),
                                Alu.add)

                    # rope: slot2 = x1*cos - x2*sin, slot3 = x1*sin + x2*cos
                    for sc in range(NSC):
                        ssl = slice(sc * SC, (sc + 1) * SC)
                        for dstT in (kT, qT):
                            x1 = dstT[:, 0, ssl]
                            x2 = dstT[:, 1, ssl]
                            t1 = ut_pool.tile([P, SC], f16, tag="ropetmp")
                            t2 = ut_pool.tile([P, SC], f16, tag="ropetmp")
                            nc.vector.tensor_mul(t1[:], x1, cos_t[:, ssl])
                            nc.vector.tensor_mul(t2[:], x2, sin_t[:, ssl])
                            nc.vector.tensor_sub(dstT[:, 2, ssl], t1[:], t2[:])
                            t3 = ut_pool.tile([P, SC], f16, tag="ropetmp")
                            t4 = ut_pool.tile([P, SC], f16, tag="ropetmp")
                            nc.vector.tensor_mul(t3[:], x1, sin_t[:, ssl])
                            nc.vector.tensor_mul(t4[:], x2, cos_t[:, ssl])
                            nc.vector.tensor_add(dstT[:, 3, ssl], t3[:], t4[:])

            # ------- phase A+O: attention with o_proj interleaved --------
            # qc-outer: once all 4 heads of a q-chunk are normalized, that
            # chunk's o_proj runs on PE underneath the next chunk's
            # ACT-bound score/exp pipeline.
            with tc.tile_pool(name="exp", bufs=2) as exp_pool, \
                 tc.tile_pool(name="wo", bufs=1) as wo_pool, \
                 tc.tile_pool(name="pss", bufs=2, space="PSUM") as pss, \
                 tc.tile_pool(name="psav", bufs=1, space="PSUM") as psav, \
                 tc.tile_pool(name="pssm", bufs=1, space="PSUM") as pssm, \
                 tc.tile_pool(name="pso", bufs=2, space="PSUM") as pso, \
                 tc.tile_pool(name="att", bufs=3) as at_pool, \
                 tc.tile_pool(name="ot", bufs=4) as ot_pool:
                # o_proj weights resident: [c-part, ct, m] layout
                wo_t = wo_pool.tile([P, NHG, H], f16)
                for ct in range(NHG):
                    nc.sync.dma_start(
                        wo_t[:, ct, :], WoT[ct * P:(ct + 1) * P, :])

                for qc in range(NSC):
                    qsl = slice(qc * SC, (qc + 1) * SC)
                    for h in range(NHG):
                        expt = exp_pool.tile([P, ST, SC], f16, tag="expt")
                        # k-tiles in pairs: two matmuls fill a 2-bank psum
                        # tile, one ACT exp covers both (amortizes the ~240ns
                        # per-ACT-instruction overhead)
                        for kth in range(ST // 2):
                            ps = pss.tile([P, 2, SC], f32, tag="score")
                            for half in (0, 1):
                                kt = 2 * kth + half
                                nc.tensor.matmul(
                                    ps[:, half, :],
                                    kT[:, h, kt * P:(kt + 1) * P],
                                    qT[:, h, qsl], start=True, stop=True)
                            nc.scalar.activation(
                                expt[:, 2 * kth:2 * kth + 2, :], ps[:],
                                Act.Exp, scale=INV_SQRT_HD)
                        # AV accumulation on PE
                        pav = psav.tile([P, SC], f32, tag="av")
                        for kt in range(ST):
                            nc.tensor.matmul(
                                pav[:], v_bf[:, kt, h * P:(h + 1) * P],
                                expt[:, kt, :],
                                start=(kt == 0), stop=(kt == ST - 1))
                        # softmax denominators: DVE-accumulate exp tiles,
                        # then one ones-matmul = 128-way partition reduce
                        # broadcast to all partitions.
                        acc = at_pool.tile([P, SC], f16, tag="acc")
                        nc.vector.tensor_add(
                            acc[:], expt[:, 0, :], expt[:, 1, :])
                        for kt in range(2, ST):
                            nc.vector.tensor_add(
                                acc[:], acc[:], expt[:, kt, :])
                        psm = pssm.tile([P, SC], f32, tag="sum")
                        nc.tensor.matmul(
                            psm[:], ones_r[:], acc[:], start=True, stop=True)
                        rec_bc = at_pool.tile([P, SC], f32, tag="rec_bc")
                        nc.vector.reciprocal_approx_fast(rec_bc[:], psm[:])
                        nc.vector.tensor_tensor(
                            attn_outT[:, h, qsl], pav[:], rec_bc[:], Alu.mult)
                    # o_proj for this q-chunk
                    for mt in range(H // P):
                        msl = slice(mt * P, (mt + 1) * P)
                        ps = pso.tile([P, SC], f32, tag="opsum")
                        for ct in range(NHG):
                            nc.tensor.matmul(
                                ps[:], wo_t[:, ct, msl],
                                attn_outT[:, ct, qsl],
                                start=(ct == 0), stop=(ct == NHG - 1))
                        ot = ot_pool.tile([P, SC], f16, tag="ot")
                        nc.vector.tensor_copy(ot[:], ps[:])
                        nc.sync.dma_start(outT[msl, qsl], ot[:])

    nc.compile()
    return nc


# ---------------------------------------------------------------------------
# Host side: shard inputs, run SPMD, gather.
# ---------------------------------------------------------------------------

def _rope_cos_sin(seq_len, dim, base=10000.0):
    inv_freq = 1.0 / (base ** (np.arange(0, dim, 2, dtype=np.float32) / dim))
    t = np.arange(seq_len, dtype=np.float32)
    freqs = np.outer(t, inv_freq).astype(np.float32)
    emb = np.concatenate([freqs, freqs], -1)
    return np.cos(emb).astype(np.float32), np.sin(emb).astype(np.float32)


def make_in_maps(hidden_states, Wq_down, bq_down, Wkv_down, bkv_down,
                 Wq_up, bq_up, Wk_up, bk_up, Wv_up, bv_up, Wo, bo):
    cos, sin = _rope_cos_sin(S, ROPE_DIM)
    WqdT = np.ascontiguousarray(Wq_down.T).astype(F16)
    WkvdT = np.ascontiguousarray(Wkv_down.T).astype(F16)
    hsT = [np.ascontiguousarray(hidden_states[b].T).astype(F16)
           for b in range(B)]
    in_maps = []
    for c in range(8):
        b, j = c // 4, c % 4
        heads = [j, 4 + j, 8 + j, 12 + j]
        x1 = slice(j * P, (j + 1) * P)
        x2 = slice(512 + j * P, 512 + (j + 1) * P)
        vrows = np.concatenate(
            [np.arange(h * P, (h + 1) * P) for h in heads])
        in_maps.append(dict(
            hsT=hsT[b],
            WqdT=WqdT, WkvdT=WkvdT,
            bqd=np.ascontiguousarray(bq_down),
            bkvd=np.ascontiguousarray(bkv_down),
            WquT=np.ascontiguousarray(
                np.concatenate([Wq_up[x1], Wq_up[x2]], 0).T),
            WkuT=np.ascontiguousarray(
                np.concatenate([Wk_up[x1], Wk_up[x2]], 0).T),
            bqku=np.stack(
                [bq_up[x1], bq_up[x2], bk_up[x1], bk_up[x2]], axis=1).copy(),
            WvuT=np.ascontiguousarray(Wv_up[vrows].T),
            bvu=np.ascontiguousarray(bv_up[vrows][None, :]),
            WoT=np.ascontiguousarray(Wo[:, vrows].T).astype(F16),
            cosT=np.ascontiguousarray(cos[:, x1].T).astype(F16),
            sinT=np.ascontiguousarray(sin[:, x1].T).astype(F16),
            ones=np.ones((P, P), np.float16),
        ))
    return in_maps


_NC_CACHE = {}


def _get_nc():
    if "nc" not in _NC_CACHE:
        _NC_CACHE["nc"] = build_mla()
    return _NC_CACHE["nc"]


LAST_RESULTS = None  # BassKernelResults of the most recent kernel() call


def kernel(**inputs):
    global LAST_RESULTS
    nc = _get_nc()
    in_maps = make_in_maps(**inputs)
    trace = bool(int(os.environ.get("MLA_TRACE", "0")))
    kwargs = {}
    if trace:
        tc_env = os.environ.get("MLA_TRACE_CORES", "0,1,2,3,4,5,6,7")
        kwargs["trace_cores"] = [int(x) for x in tc_env.split(",")]
    res = run_bass_kernel_spmd(
        nc, in_maps, core_ids=list(range(8)), trace=trace, **kwargs)
    LAST_RESULTS = res
    bo = inputs["bo"]
    out = np.zeros((B, S, H), np.float32)
    for b in range(B):
        acc = res.results[b * 4]["outT"].astype(np.float32)
        for j in range(1, 4):
            acc = acc + res.results[b * 4 + j]["outT"]
        out[b] = acc.T + bo[None, :]
    return out


# revision 30
# speedup vs baseline: 1.0451x; 1.0300x over previous
"""Trainium2 Bass kernel for MultiHeadLatentAttention (B=2, S=2048, H=2048,
NH=16, HD=128, LAT=512), SPMD across 8 NeuronCores.

Sharding: 8 cores = 2 (batch) x 4 (head-group TP). Core c handles batch c//4
and head group j = c%4 = heads {j, 4+j, 8+j, 12+j}. That grouping is chosen so
the 4 heads share exactly 256 rows of Wq_up/Wk_up: heads j and 4+j are the raw
x1/x2 slices of q_half, heads 8+j and 12+j are their RoPE combinations — so
the up-projection shards 4-way with no duplication. Each core computes its
partial o_proj output; the host sums the 4 partials per batch and adds bo.

Self-contained: builds + compiles the Bass program on first call (cached),
runs via run_bass_kernel_spmd on cores 0-7.
"""
import os
import sys
import types
from contextlib import ExitStack

import numpy as np

if "/opt/trn_rl_repo" not in sys.path:
    sys.path.insert(0, "/opt/trn_rl_repo")

import ml_dtypes

# ---------------------------------------------------------------------------
# NTFF-profile shim: antenv.axon_hooks is missing in this image; register a
# hook backed by the axon PJRT .so so trace=True can capture HW exec time.
# ---------------------------------------------------------------------------


def _install_axon_hooks_shim():
    if "antenv.axon_hooks" in sys.modules:
        return
    try:
        import antenv
        from trn_agent_boot.trn_boot import _ntff_profile_via_ctypes
        hook = _ntff_profile_via_ctypes("/opt/axon/libaxon_pjrt.so")
    except Exception:
        return
    mod = types.ModuleType("antenv.axon_hooks")
    mod.get_axon_ntff_profile_hook = lambda: hook
    mod.set_axon_ntff_profile_hook = lambda h: None
    sys.modules["antenv.axon_hooks"] = mod
    antenv.axon_hooks = mod


_install_axon_hooks_shim()

import concourse.bass as bass  # noqa: E402
import concourse.mybir as mybir  # noqa: E402
import concourse.tile as tile  # noqa: E402
from concourse import bacc  # noqa: E402
from concourse.bass_utils import run_bass_kernel_spmd  # noqa: E402

P = 128
H = 2048
NH = 16
HD = 128
LAT = 512
B = 2
S = 2048
ROPE_DIM = H // 4
NHG = 4          # heads per core
SC = 512         # s/q chunk (one PSUM bank of fp32)
INV_SQRT_HD = 0.08838834764831845  # 1/sqrt(128)

f32 = mybir.dt.float32
f32r = mybir.dt.float32r
bf16 = mybir.dt.bfloat16
f16 = mybir.dt.float16
Act = mybir.ActivationFunctionType
Alu = mybir.AluOpType
BF16 = ml_dtypes.bfloat16
F16 = np.float16


def build_mla(seq=S, debug=False):
    """Build one core's program. All cores run this same program SPMD."""
    NSC = seq // SC   # s-chunks
    HT = H // P       # 16 h-tiles
    LT = LAT // P     # 4 l-tiles
    ST = seq // P     # s-tiles (= k-tiles in attention)

    nc = bacc.Bacc("TRN2", target_bir_lowering=False, debug=debug)

    hsT = nc.dram_tensor("hsT", [H, seq], f16, kind="ExternalInput")
    WqdT = nc.dram_tensor("WqdT", [H, LAT], f16, kind="ExternalInput")
    WkvdT = nc.dram_tensor("WkvdT", [H, LAT], f16, kind="ExternalInput")
    bqd = nc.dram_tensor("bqd", [LAT], f32, kind="ExternalInput")
    bkvd = nc.dram_tensor("bkvd", [LAT], f32, kind="ExternalInput")
    WquT = nc.dram_tensor("WquT", [LAT, 2 * P], f32r, kind="ExternalInput")
    WkuT = nc.dram_tensor("WkuT", [LAT, 2 * P], f32r, kind="ExternalInput")
    bqku = nc.dram_tensor("bqku", [P, 4], f32, kind="ExternalInput")
    WvuT = nc.dram_tensor("WvuT", [LAT, NHG * P], f32r, kind="ExternalInput")
    bvu = nc.dram_tensor("bvu", [1, NHG * P], f32, kind="ExternalInput")
    WoT = nc.dram_tensor("WoT", [NHG * P, H], f16, kind="ExternalInput")
    cosT = nc.dram_tensor("cosT", [P, seq], f16, kind="ExternalInput")
    sinT = nc.dram_tensor("sinT", [P, seq], f16, kind="ExternalInput")
    ones = nc.dram_tensor("ones", [P, P], f16, kind="ExternalInput")
    outT = nc.dram_tensor("outT", [H, seq], f16, kind="ExternalOutput")

    def r(ap):  # fast fp32 matmul path
        return ap.bitcast(f32r)

    with tile.TileContext(nc) as tc, ExitStack() as top:
        const = top.enter_context(tc.tile_pool(name="const", bufs=1))
        ao_pool = top.enter_context(tc.tile_pool(name="ao", bufs=1))

        bqd_t = const.tile([P, LT], f32)
        nc.sync.dma_start(bqd_t[:], bqd.rearrange("(o p) -> p o", p=P))
        bkvd_t = const.tile([P, LT], f32)
        nc.sync.dma_start(bkvd_t[:], bkvd.rearrange("(o p) -> p o", p=P))
        ones_r = const.tile([P, P], f16)
        nc.sync.dma_start(ones_r[:], ones[:])

        # HAM warmup: ~64 back-to-back matmuls (~3.5us of PE activity) while
        # the initial weight/activation DMAs stream in, so the first real
        # matmuls run at 2.4GHz instead of the cold 1.2GHz.
        with tc.tile_pool(name="warm", bufs=1, space="PSUM") as warm_pool:
            wtiles = [warm_pool.tile([P, P], f32, tag=f"w{i}", name=f"warm{i}")
                      for i in range(4)]
            for i in range(48):
                nc.tensor.matmul(wtiles[i % 4][:], ones_r[:], ones_r[:],
                                 start=True, stop=True)

        attn_outT = ao_pool.tile([P, NHG, seq], f16)

        with ExitStack() as qkv_scope:
            qk_pool = qkv_scope.enter_context(tc.tile_pool(name="qk", bufs=1))
            v_pool = qkv_scope.enter_context(tc.tile_pool(name="v", bufs=1))
            qT = qk_pool.tile([P, NHG, seq], f16)  # 0=x1, 1=x2, 2,3=rope
            kT = qk_pool.tile([P, NHG, seq], f16)
            v_bf = v_pool.tile([P, ST, NHG * P], f16)  # token-major v

            with ExitStack() as lat_scope:
                lat_pool = lat_scope.enter_context(
                    tc.tile_pool(name="lat", bufs=1))
                q_latT = lat_pool.tile([P, LT, seq], f32r)
                kv_latT = lat_pool.tile([P, LT, seq], f32r)

                # ---------------- phase D: down projections ----------------
                with tc.tile_pool(name="wd", bufs=1) as wd_pool, \
                     tc.tile_pool(name="hst", bufs=HT + 12) as hst_pool, \
                     tc.tile_pool(name="psd", bufs=4, space="PSUM") as psd:
                    # per-ht DMA split, interleaved with the first s-chunk's
                    # activation tiles, so matmul (m=0, ht=0) can start after
                    # ~400KB of DMA instead of the full 12MB initial load
                    # load order: first s-chunk of activations, then Wq_down
                    # (chains m=0..3 need it), then Wkv_down (m=4..7) — the
                    # first chain starts as soon as ~4MB has landed.
                    wqd_t = wd_pool.tile([P, HT, LAT], f16)
                    wkvd_t = wd_pool.tile([P, HT, LAT], f16)
                    hts0 = []
                    for ht in range(HT):
                        t = hst_pool.tile([P, SC], f16, tag="hst")
                        nc.sync.dma_start(t[:], hsT[ht * P:(ht + 1) * P, :SC])
                        hts0.append(t)
                    for ht in range(HT):
                        nc.sync.dma_start(
                            wqd_t[:, ht, :], WqdT[ht * P:(ht + 1) * P, :])
                    for ht in range(HT):
                        nc.sync.dma_start(
                            wkvd_t[:, ht, :], WkvdT[ht * P:(ht + 1) * P, :])

                    for sc in range(NSC):
                        ssl = slice(sc * SC, (sc + 1) * SC)
                        if sc == 0:
                            hts = hts0
                        else:
                            hts = []
                            for ht in range(HT):
                                t = hst_pool.tile([P, SC], f16, tag="hst")
                                nc.sync.dma_start(
                                    t[:], hsT[ht * P:(ht + 1) * P, ssl])
                                hts.append(t)
                        for m in range(2 * LT):
                            w_t, dst, b_t = ((wqd_t, q_latT, bqd_t) if m < LT
                                             else (wkvd_t, kv_latT, bkvd_t))
                            lt = m % LT
                            ps = psd.tile([P, SC], f32)
                            for ht in range(HT):
                                nc.tensor.matmul(
                                    ps[:],
                                    w_t[:, ht, lt * P:(lt + 1) * P],
                                    hts[ht][:],
                                    start=(ht == 0), stop=(ht == HT - 1))
                            nc.scalar.activation(
                                dst[:, lt, ssl], ps[:], Act.Identity,
                                bias=b_t[:, lt:lt + 1])

                # ------------- phase U: up projections + rope --------------
                with tc.tile_pool(name="wu", bufs=1) as wu_pool, \
                     tc.tile_pool(name="psu", bufs=4, space="PSUM") as psu, \
                     tc.tile_pool(name="ut", bufs=4) as ut_pool:
                    bqku_t = wu_pool.tile([P, 4], f32)
                    nc.sync.dma_start(bqku_t[:], bqku[:])
                    bvu_bc = wu_pool.tile([P, NHG * P], f32)
                    nc.sync.dma_start(
                        bvu_bc[:], bvu[:].to_broadcast((P, NHG * P)))
                    cos_t = wu_pool.tile([P, seq], f16)
                    nc.sync.dma_start(cos_t[:], cosT[:])
                    sin_t = wu_pool.tile([P, seq], f16)
                    nc.sync.dma_start(sin_t[:], sinT[:])
                    wqu_t = wu_pool.tile([P, LT, 2 * P], f32r)
                    nc.sync.dma_start(
                        wqu_t[:], WquT.rearrange("(lt p) m -> p lt m", p=P))
                    wku_t = wu_pool.tile([P, LT, 2 * P], f32r)
                    nc.sync.dma_start(
                        wku_t[:], WkuT.rearrange("(lt p) m -> p lt m", p=P))
                    wvu_t = wu_pool.tile([P, LT, NHG * P], f32r)
                    nc.sync.dma_start(
                        wvu_t[:], WvuT.rearrange("(lt p) m -> p lt m", p=P))

                    # v first (every AV chain in phase A needs ALL v tiles),
                    # then k_x1/q_x1 before x2 so attention head 0 can start
                    # while the rest of phase U still runs.
                    for st in range(ST):
                        ps = psu.tile([P, NHG * P], f32, tag="psu_v")
                        for lt in range(LT):
                            nc.tensor.matmul(
                                ps[:], kv_latT[:, lt, st * P:(st + 1) * P],
                                wvu_t[:, lt, :],
                                start=(lt == 0), stop=(lt == LT - 1))
                        nc.vector.tensor_tensor(
                            v_bf[:, st, :], ps[:], bvu_bc[:], Alu.add)

                    for ci in (2, 0, 3, 1):  # k_x1, q_x1, k_x2, q_x2
                        w_t = wqu_t if ci < 2 else wku_t
                        latsrc = q_latT if ci < 2 else kv_latT
                        csl = slice((ci % 2) * P, (ci % 2) * P + P)
                        dstT = qT if ci < 2 else kT
                        for sc in range(NSC):
                            ssl = slice(sc * SC, (sc + 1) * SC)
                            ps = psu.tile([P, SC], f32, tag="psu_qk")
                            for lt in range(LT):
                                nc.tensor.matmul(
                                    ps[:], w_t[:, lt, csl],
                                    latsrc[:, lt, ssl],
                                    start=(lt == 0), stop=(lt == LT - 1))
                            # bias-add on DVE (free-dim broadcast of [P,1])
                            # to keep ACT free for the attention exps
                            nc.vector.tensor_tensor(
                                dstT[:, ci % 2, ssl], ps[:],
                                bqku_t[:, ci:ci + 1].to_broadcast((P, SC)),
                                Alu.add)

                    # rope: slot2 = x1*cos - x2*sin, slot3 = x1*sin + x2*cos
                    for sc in range(NSC):
                        ssl = slice(sc * SC, (sc + 1) * SC)
                        for dstT in (kT, qT):
                            x1 = dstT[:, 0, ssl]
                            x2 = dstT[:, 1, ssl]
                            t1 = ut_pool.tile([P, SC], f16, tag="ropetmp")
                            t2 = ut_pool.tile([P, SC], f16, tag="ropetmp")
                            nc.vector.tensor_mul(t1[:], x1, cos_t[:, ssl])
                            nc.vector.tensor_mul(t2[:], x2, sin_t[:, ssl])
                            nc.vector.tensor_sub(dstT[:, 2, ssl], t1[:], t2[:])
                            t3 = ut_pool.tile([P, SC], f16, tag="ropetmp")
                            t4 = ut_pool.tile([P, SC], f16, tag="ropetmp")
                            nc.vector.tensor_mul(t3[:], x1, sin_t[:, ssl])
                            nc.vector.tensor_mul(t4[:], x2, cos_t[:, ssl])
                            nc.vector.tensor_add(dstT[:, 3, ssl], t3[:], t4[:])

            # ------- phase A+O: attention with o_proj interleaved --------
            # qc-outer: once all 4 heads of a q-chunk are normalized, that
            # chunk's o_proj runs on PE underneath the next chunk's
            # ACT-bound score/exp pipeline.
            with tc.tile_pool(name="exp", bufs=2) as exp_pool, \
                 tc.tile_pool(name="wo", bufs=1) as wo_pool, \
                 tc.tile_pool(name="pss", bufs=2, space="PSUM") as pss, \
                 tc.tile_pool(name="psav", bufs=1, space="PSUM") as psav, \
                 tc.tile_pool(name="pssm", bufs=1, space="PSUM") as pssm, \
                 tc.tile_pool(name="pso", bufs=2, space="PSUM") as pso, \
                 tc.tile_pool(name="att", bufs=3) as at_pool, \
                 tc.tile_pool(name="ot", bufs=4) as ot_pool:
                # o_proj weights resident: [c-part, ct, m] layout
                wo_t = wo_pool.tile([P, NHG, H], f16)
                for ct in range(NHG):
                    nc.sync.dma_start(
                        wo_t[:, ct, :], WoT[ct * P:(ct + 1) * P, :])

                for qc in range(NSC):
                    qsl = slice(qc * SC, (qc + 1) * SC)
                    for h in range(NHG):
                        expt = exp_pool.tile([P, ST, SC], f16, tag="expt")
                        # k-tiles in pairs: two matmuls fill a 2-bank psum
                        # tile, one ACT exp covers both (amortizes the ~240ns
                        # per-ACT-instruction overhead)
                        for kth in range(ST // 2):
                            ps = pss.tile([P, 2, SC], f32, tag="score")
                            for half in (0, 1):
                                kt = 2 * kth + half
                                nc.tensor.matmul(
                                    ps[:, half, :],
                                    kT[:, h, kt * P:(kt + 1) * P],
                                    qT[:, h, qsl], start=True, stop=True)
                            nc.scalar.activation(
                                expt[:, 2 * kth:2 * kth + 2, :], ps[:],
                                Act.Exp, scale=INV_SQRT_HD)
                        # AV accumulation on PE
                        pav = psav.tile([P, SC], f32, tag="av")
                        for kt in range(ST):
                            nc.tensor.matmul(
                                pav[:], v_bf[:, kt, h * P:(h + 1) * P],
                                expt[:, kt, :],
                                start=(kt == 0), stop=(kt == ST - 1))
                        # softmax denominators: DVE-accumulate exp tiles,
                        # then one ones-matmul = 128-way partition reduce
                        # broadcast to all partitions.
                        acc = at_pool.tile([P, SC], f16, tag="acc")
                        nc.vector.tensor_add(
                            acc[:], expt[:, 0, :], expt[:, 1, :])
                        for kt in range(2, ST):
                            nc.vector.tensor_add(
                                acc[:], acc[:], expt[:, kt, :])
                        psm = pssm.tile([P, SC], f32, tag="sum")
                        nc.tensor.matmul(
                            psm[:], ones_r[:], acc[:], start=True, stop=True)
                        rec_bc = at_pool.tile([P, SC], f32, tag="rec_bc")
                        nc.vector.reciprocal_approx_fast(rec_bc[:], psm[:])
                        nc.vector.tensor_tensor(
                            attn_outT[:, h, qsl], pav[:], rec_bc[:], Alu.mult)
                    # o_proj for this q-chunk
                    for mt in range(H // P):
                        msl = slice(mt * P, (mt + 1) * P)
                        ps = pso.tile([P, SC], f32, tag="opsum")
                        for ct in range(NHG):
                            nc.tensor.matmul(
                                ps[:], wo_t[:, ct, msl],
                                attn_outT[:, ct, qsl],
                                start=(ct == 0), stop=(ct == NHG - 1))
                        ot = ot_pool.tile([P, SC], f16, tag="ot")
                        nc.vector.tensor_copy(ot[:], ps[:])
                        nc.sync.dma_start(outT[msl, qsl], ot[:])

    nc.compile()
    return nc


# ---------------------------------------------------------------------------
# Host side: shard inputs, run SPMD, gather.
# ---------------------------------------------------------------------------

def _rope_cos_sin(seq_len, dim, base=10000.0):
    inv_freq = 1.0 / (base ** (np.arange(0, dim, 2, dtype=np.float32) / dim))
    t = np.arange(seq_len, dtype=np.float32)
    freqs = np.outer(t, inv_freq).astype(np.float32)
    emb = np.concatenate([freqs, freqs], -1)
    return np.cos(emb).astype(np.float32), np.sin(emb).astype(np.float32)


def make_in_maps(hidden_states, Wq_down, bq_down, Wkv_down, bkv_down,
                 Wq_up, bq_up, Wk_up, bk_up, Wv_up, bv_up, Wo, bo):
    cos, sin = _rope_cos_sin(S, ROPE_DIM)
    WqdT = np.ascontiguousarray(Wq_down.T).astype(F16)
    WkvdT = np.ascontiguousarray(Wkv_down.T).astype(F16)
    hsT = [np.ascontiguousarray(hidden_states[b].T).astype(F16)
           for b in range(B)]
    in_maps = []
    for c in range(8):
        b, j = c // 4, c % 4
        heads = [j, 4 + j, 8 + j, 12 + j]
        x1 = slice(j * P, (j + 1) * P)
        x2 = slice(512 + j * P, 512 + (j + 1) * P)
        vrows = np.concatenate(
            [np.arange(h * P, (h + 1) * P) for h in heads])
        in_maps.append(dict(
            hsT=hsT[b],
            WqdT=WqdT, WkvdT=WkvdT,
            bqd=np.ascontiguousarray(bq_down),
            bkvd=np.ascontiguousarray(bkv_down),
            WquT=np.ascontiguousarray(
                np.concatenate([Wq_up[x1], Wq_up[x2]], 0).T),
            WkuT=np.ascontiguousarray(
                np.concatenate([Wk_up[x1], Wk_up[x2]], 0).T),
            bqku=np.stack(
                [bq_up[x1], bq_up[x2], bk_up[x1], bk_up[x2]], axis=1).copy(),
            WvuT=np.ascontiguousarray(Wv_up[vrows].T),
            bvu=np.ascontiguousarray(bv_up[vrows][None, :]),
            WoT=np.ascontiguousarray(Wo[:, vrows].T).astype(F16),
            cosT=np.ascontiguousarray(cos[:, x1].T).astype(F16),
            sinT=np.ascontiguousarray(sin[:, x1].T).astype(F16),
            ones=np.ones((P, P), np.float16),
        ))
    return in_maps


_NC_CACHE = {}


def _get_nc():
    if "nc" not in _NC_CACHE:
        _NC_CACHE["nc"] = build_mla()
    return _NC_CACHE["nc"]


LAST_RESULTS = None  # BassKernelResults of the most recent kernel() call


def kernel(**inputs):
    global LAST_RESULTS
    nc = _get_nc()
    in_maps = make_in_maps(**inputs)
    trace = bool(int(os.environ.get("MLA_TRACE", "0")))
    kwargs = {}
    if trace:
        tc_env = os.environ.get("MLA_TRACE_CORES", "0,1,2,3,4,5,6,7")
        kwargs["trace_cores"] = [int(x) for x in tc_env.split(",")]
    res = run_bass_kernel_spmd(
        nc, in_maps, core_ids=list(range(8)), trace=trace, **kwargs)
    LAST_RESULTS = res
    bo = inputs["bo"]
    out = np.zeros((B, S, H), np.float32)
    for b in range(B):
        acc = res.results[b * 4]["outT"].astype(np.float32)
        for j in range(1, 4):
            acc = acc + res.results[b * 4 + j]["outT"]
        out[b] = acc.T + bo[None, :]
    return out


# revision 33
# speedup vs baseline: 1.0545x; 1.0090x over previous
"""Trainium2 Bass kernel for MultiHeadLatentAttention (B=2, S=2048, H=2048,
NH=16, HD=128, LAT=512), SPMD across 8 NeuronCores.

Sharding: 8 cores = 2 (batch) x 4 (head-group TP). Core c handles batch c//4
and head group j = c%4 = heads {j, 4+j, 8+j, 12+j}. That grouping is chosen so
the 4 heads share exactly 256 rows of Wq_up/Wk_up: heads j and 4+j are the raw
x1/x2 slices of q_half, heads 8+j and 12+j are their RoPE combinations — so
the up-projection shards 4-way with no duplication. Each core computes its
partial o_proj output; the host sums the 4 partials per batch and adds bo.

Self-contained: builds + compiles the Bass program on first call (cached),
runs via run_bass_kernel_spmd on cores 0-7.
"""
import os
import sys
import types
from contextlib import ExitStack

import numpy as np

if "/opt/trn_rl_repo" not in sys.path:
    sys.path.insert(0, "/opt/trn_rl_repo")

import ml_dtypes

# ---------------------------------------------------------------------------
# NTFF-profile shim: antenv.axon_hooks is missing in this image; register a
# hook backed by the axon PJRT .so so trace=True can capture HW exec time.
# ---------------------------------------------------------------------------


def _install_axon_hooks_shim():
    if "antenv.axon_hooks" in sys.modules:
        return
    try:
        import antenv
        from trn_agent_boot.trn_boot import _ntff_profile_via_ctypes
        hook = _ntff_profile_via_ctypes("/opt/axon/libaxon_pjrt.so")
    except Exception:
        return
    mod = types.ModuleType("antenv.axon_hooks")
    mod.get_axon_ntff_profile_hook = lambda: hook
    mod.set_axon_ntff_profile_hook = lambda h: None
    sys.modules["antenv.axon_hooks"] = mod
    antenv.axon_hooks = mod


_install_axon_hooks_shim()

import concourse.bass as bass  # noqa: E402
import concourse.mybir as mybir  # noqa: E402
import concourse.tile as tile  # noqa: E402
from concourse import bacc  # noqa: E402
from concourse.bass_utils import run_bass_kernel_spmd  # noqa: E402

P = 128
H = 2048
NH = 16
HD = 128
LAT = 512
B = 2
S = 2048
ROPE_DIM = H // 4
NHG = 4          # heads per core
SC = 512         # s/q chunk (one PSUM bank of fp32)
INV_SQRT_HD = 0.08838834764831845  # 1/sqrt(128)

f32 = mybir.dt.float32
f32r = mybir.dt.float32r
bf16 = mybir.dt.bfloat16
f16 = mybir.dt.float16
Act = mybir.ActivationFunctionType
Alu = mybir.AluOpType
BF16 = ml_dtypes.bfloat16
F16 = np.float16


def build_mla(seq=S, debug=False):
    """Build one core's program. All cores run this same program SPMD."""
    NSC = seq // SC   # s-chunks
    HT = H // P       # 16 h-tiles
    LT = LAT // P     # 4 l-tiles
    ST = seq // P     # s-tiles (= k-tiles in attention)

    nc = bacc.Bacc("TRN2", target_bir_lowering=False, debug=debug)

    hsT = nc.dram_tensor("hsT", [H, seq], f16, kind="ExternalInput")
    WqdT = nc.dram_tensor("WqdT", [H, LAT], f16, kind="ExternalInput")
    WkvdT = nc.dram_tensor("WkvdT", [H, LAT], f16, kind="ExternalInput")
    bqd = nc.dram_tensor("bqd", [LAT], f32, kind="ExternalInput")
    bkvd = nc.dram_tensor("bkvd", [LAT], f32, kind="ExternalInput")
    WquT = nc.dram_tensor("WquT", [LAT, 2 * P], f32r, kind="ExternalInput")
    WkuT = nc.dram_tensor("WkuT", [LAT, 2 * P], f32r, kind="ExternalInput")
    bqku = nc.dram_tensor("bqku", [P, 4], f32, kind="ExternalInput")
    WvuT = nc.dram_tensor("WvuT", [LAT, NHG * P], f32r, kind="ExternalInput")
    bvu = nc.dram_tensor("bvu", [1, NHG * P], f32, kind="ExternalInput")
    WoT = nc.dram_tensor("WoT", [NHG * P, H], f16, kind="ExternalInput")
    cosT = nc.dram_tensor("cosT", [P, seq], f16, kind="ExternalInput")
    sinT = nc.dram_tensor("sinT", [P, seq], f16, kind="ExternalInput")
    ones = nc.dram_tensor("ones", [P, P], f16, kind="ExternalInput")
    outT = nc.dram_tensor("outT", [H, seq], f16, kind="ExternalOutput")

    def r(ap):  # fast fp32 matmul path
        return ap.bitcast(f32r)

    with tile.TileContext(nc) as tc, ExitStack() as top:
        const = top.enter_context(tc.tile_pool(name="const", bufs=1))
        ao_pool = top.enter_context(tc.tile_pool(name="ao", bufs=1))

        bqd_t = const.tile([P, LT], f32)
        nc.sync.dma_start(bqd_t[:], bqd.rearrange("(o p) -> p o", p=P))
        bkvd_t = const.tile([P, LT], f32)
        nc.sync.dma_start(bkvd_t[:], bkvd.rearrange("(o p) -> p o", p=P))
        ones_r = const.tile([P, P], f16)
        nc.sync.dma_start(ones_r[:], ones[:])

        # HAM warmup: ~64 back-to-back matmuls (~3.5us of PE activity) while
        # the initial weight/activation DMAs stream in, so the first real
        # matmuls run at 2.4GHz instead of the cold 1.2GHz.
        with tc.tile_pool(name="warm", bufs=1, space="PSUM") as warm_pool:
            wtiles = [warm_pool.tile([P, P], f32, tag=f"w{i}", name=f"warm{i}")
                      for i in range(4)]
            for i in range(48):
                nc.tensor.matmul(wtiles[i % 4][:], ones_r[:], ones_r[:],
                                 start=True, stop=True)

        attn_outT = ao_pool.tile([P, NHG, seq], f16)

        with ExitStack() as qkv_scope:
            qk_pool = qkv_scope.enter_context(tc.tile_pool(name="qk", bufs=1))
            v_pool = qkv_scope.enter_context(tc.tile_pool(name="v", bufs=1))
            qT = qk_pool.tile([P, NHG, seq], f16)  # 0=x1, 1=x2, 2,3=rope
            kT = qk_pool.tile([P, NHG, seq], f16)
            v_bf = v_pool.tile([P, ST, NHG * P], f16)  # token-major v

            with ExitStack() as lat_scope:
                lat_pool = lat_scope.enter_context(
                    tc.tile_pool(name="lat", bufs=1))
                q_latT = lat_pool.tile([P, LT, seq], f32r)
                kv_latT = lat_pool.tile([P, LT, seq], f32r)
                # early-U pool + U psum hoisted ABOVE the D pools: v-up can
                # start the moment the last D chain drains, instead of
                # waiting for D's pools to release and ~2.5MB of U DMAs.
                ue_pool = lat_scope.enter_context(
                    tc.tile_pool(name="uearly", bufs=1))
                psu = lat_scope.enter_context(
                    tc.tile_pool(name="psu", bufs=4, space="PSUM"))
                bqku_t = ue_pool.tile([P, 4], f32)
                nc.sync.dma_start(bqku_t[:], bqku[:])
                bvu_bc = ue_pool.tile([P, NHG * P], f32)
                nc.sync.dma_start(
                    bvu_bc[:], bvu[:].to_broadcast((P, NHG * P)))
                wvu_t = ue_pool.tile([P, LT, NHG * P], f32r)
                nc.sync.dma_start(
                    wvu_t[:], WvuT.rearrange("(lt p) m -> p lt m", p=P))

                # ---------------- phase D: down projections ----------------
                with tc.tile_pool(name="wd", bufs=1) as wd_pool, \
                     tc.tile_pool(name="hst", bufs=HT + 4) as hst_pool, \
                     tc.tile_pool(name="psd", bufs=4, space="PSUM") as psd:
                    # per-ht DMA split, interleaved with the first s-chunk's
                    # activation tiles, so matmul (m=0, ht=0) can start after
                    # ~400KB of DMA instead of the full 12MB initial load
                    # load order: first s-chunk of activations, then Wq_down
                    # (chains m=0..3 need it), then Wkv_down (m=4..7) — the
                    # first chain starts as soon as ~4MB has landed.
                    wqd_t = wd_pool.tile([P, HT, LAT], f16)
                    wkvd_t = wd_pool.tile([P, HT, LAT], f16)
                    hts0 = []
                    for ht in range(HT):
                        t = hst_pool.tile([P, SC], f16, tag="hst")
                        nc.sync.dma_start(t[:], hsT[ht * P:(ht + 1) * P, :SC])
                        hts0.append(t)
                    for ht in range(HT):
                        nc.sync.dma_start(
                            wqd_t[:, ht, :], WqdT[ht * P:(ht + 1) * P, :])
                    for ht in range(HT):
                        nc.sync.dma_start(
                            wkvd_t[:, ht, :], WkvdT[ht * P:(ht + 1) * P, :])

                    for sc in range(NSC):
                        ssl = slice(sc * SC, (sc + 1) * SC)
                        if sc == 0:
                            hts = hts0
                        else:
                            hts = []
                            for ht in range(HT):
                                t = hst_pool.tile([P, SC], f16, tag="hst")
                                nc.sync.dma_start(
                                    t[:], hsT[ht * P:(ht + 1) * P, ssl])
                                hts.append(t)
                        for m in range(2 * LT):
                            w_t, dst, b_t = ((wqd_t, q_latT, bqd_t) if m < LT
                                             else (wkvd_t, kv_latT, bkvd_t))
                            lt = m % LT
                            ps = psd.tile([P, SC], f32)
                            for ht in range(HT):
                                nc.tensor.matmul(
                                    ps[:],
                                    w_t[:, ht, lt * P:(lt + 1) * P],
                                    hts[ht][:],
                                    start=(ht == 0), stop=(ht == HT - 1))
                            nc.scalar.activation(
                                dst[:, lt, ssl], ps[:], Act.Identity,
                                bias=b_t[:, lt:lt + 1])

                # ------------- phase U: up projections + rope --------------
                with tc.tile_pool(name="wu", bufs=1) as wu_pool, \
                     tc.tile_pool(name="ut", bufs=4) as ut_pool:
                    cos_t = wu_pool.tile([P, seq], f16)
                    nc.sync.dma_start(cos_t[:], cosT[:])
                    sin_t = wu_pool.tile([P, seq], f16)
                    nc.sync.dma_start(sin_t[:], sinT[:])
                    wqu_t = wu_pool.tile([P, LT, 2 * P], f32r)
                    nc.sync.dma_start(
                        wqu_t[:], WquT.rearrange("(lt p) m -> p lt m", p=P))
                    wku_t = wu_pool.tile([P, LT, 2 * P], f32r)
                    nc.sync.dma_start(
                        wku_t[:], WkuT.rearrange("(lt p) m -> p lt m", p=P))

                    # v first (every AV chain in phase A needs ALL v tiles),
                    # then k_x1/q_x1 before x2 so attention head 0 can start
                    # while the rest of phase U still runs.
                    for st in range(ST):
                        ps = psu.tile([P, NHG * P], f32, tag="psu")
                        for lt in range(LT):
                            nc.tensor.matmul(
                                ps[:], kv_latT[:, lt, st * P:(st + 1) * P],
                                wvu_t[:, lt, :],
                                start=(lt == 0), stop=(lt == LT - 1))
                        nc.vector.tensor_tensor(
                            v_bf[:, st, :], ps[:], bvu_bc[:], Alu.add)

                    for ci in (2, 0, 3, 1):  # k_x1, q_x1, k_x2, q_x2
                        w_t = wqu_t if ci < 2 else wku_t
                        latsrc = q_latT if ci < 2 else kv_latT
                        csl = slice((ci % 2) * P, (ci % 2) * P + P)
                        dstT = qT if ci < 2 else kT
                        for sc in range(NSC):
                            ssl = slice(sc * SC, (sc + 1) * SC)
                            ps = psu.tile([P, SC], f32, tag="psu")
                            for lt in range(LT):
                                nc.tensor.matmul(
                                    ps[:], w_t[:, lt, csl],
                                    latsrc[:, lt, ssl],
                                    start=(lt == 0), stop=(lt == LT - 1))
                            # bias-add on DVE (free-dim broadcast of [P,1])
                            # to keep ACT free for the attention exps
                            nc.vector.tensor_tensor(
                                dstT[:, ci % 2, ssl], ps[:],
                                bqku_t[:, ci:ci + 1].to_broadcast((P, SC)),
                                Alu.add)

                    # rope: slot2 = x1*cos - x2*sin, slot3 = x1*sin + x2*cos
                    for sc in range(NSC):
                        ssl = slice(sc * SC, (sc + 1) * SC)
                        for dstT in (kT, qT):
                            x1 = dstT[:, 0, ssl]
                            x2 = dstT[:, 1, ssl]
                            t1 = ut_pool.tile([P, SC], f16, tag="ropetmp")
                            t2 = ut_pool.tile([P, SC], f16, tag="ropetmp")
                            nc.vector.tensor_mul(t1[:], x1, cos_t[:, ssl])
                            nc.vector.tensor_mul(t2[:], x2, sin_t[:, ssl])
                            nc.vector.tensor_sub(dstT[:, 2, ssl], t1[:], t2[:])
                            t3 = ut_pool.tile([P, SC], f16, tag="ropetmp")
                            t4 = ut_pool.tile([P, SC], f16, tag="ropetmp")
                            nc.vector.tensor_mul(t3[:], x1, sin_t[:, ssl])
                            nc.vector.tensor_mul(t4[:], x2, cos_t[:, ssl])
                            nc.vector.tensor_add(dstT[:, 3, ssl], t3[:], t4[:])

            # ------- phase A+O: attention with o_proj interleaved --------
            # qc-outer: once all 4 heads of a q-chunk are normalized, that
            # chunk's o_proj runs on PE underneath the next chunk's
            # ACT-bound score/exp pipeline.
            with tc.tile_pool(name="exp", bufs=2) as exp_pool, \
                 tc.tile_pool(name="wo", bufs=1) as wo_pool, \
                 tc.tile_pool(name="pss", bufs=2, space="PSUM") as pss, \
                 tc.tile_pool(name="psav", bufs=1, space="PSUM") as psav, \
                 tc.tile_pool(name="pssm", bufs=1, space="PSUM") as pssm, \
                 tc.tile_pool(name="pso", bufs=2, space="PSUM") as pso, \
                 tc.tile_pool(name="att", bufs=3) as at_pool, \
                 tc.tile_pool(name="ot", bufs=4) as ot_pool:
                # o_proj weights resident: [c-part, ct, m] layout
                wo_t = wo_pool.tile([P, NHG, H], f16)
                for ct in range(NHG):
                    nc.sync.dma_start(
                        wo_t[:, ct, :], WoT[ct * P:(ct + 1) * P, :])

                for qc in range(NSC):
                    qsl = slice(qc * SC, (qc + 1) * SC)
                    for h in range(NHG):
                        expt = exp_pool.tile([P, ST, SC], f16, tag="expt")
                        # k-tiles in pairs: two matmuls fill a 2-bank psum
                        # tile, one ACT exp covers both (amortizes the ~240ns
                        # per-ACT-instruction overhead)
                        for kth in range(ST // 2):
                            ps = pss.tile([P, 2, SC], f32, tag="score")
                            for half in (0, 1):
                                kt = 2 * kth + half
                                nc.tensor.matmul(
                                    ps[:, half, :],
                                    kT[:, h, kt * P:(kt + 1) * P],
                                    qT[:, h, qsl], start=True, stop=True)
                            nc.scalar.activation(
                                expt[:, 2 * kth:2 * kth + 2, :], ps[:],
                                Act.Exp, scale=INV_SQRT_HD)
                        # AV accumulation on PE
                        pav = psav.tile([P, SC], f32, tag="av")
                        for kt in range(ST):
                            nc.tensor.matmul(
                                pav[:], v_bf[:, kt, h * P:(h + 1) * P],
                                expt[:, kt, :],
                                start=(kt == 0), stop=(kt == ST - 1))
                        # softmax denominators: DVE-accumulate exp tiles,
                        # then one ones-matmul = 128-way partition reduce
                        # broadcast to all partitions.
                        acc = at_pool.tile([P, SC], f16, tag="acc")
                        nc.vector.tensor_add(
                            acc[:], expt[:, 0, :], expt[:, 1, :])
                        for kt in range(2, ST):
                            nc.vector.tensor_add(
                                acc[:], acc[:], expt[:, kt, :])
                        psm = pssm.tile([P, SC], f32, tag="sum")
                        nc.tensor.matmul(
                            psm[:], ones_r[:], acc[:], start=True, stop=True)
                        rec_bc = at_pool.tile([P, SC], f32, tag="rec_bc")
                        nc.vector.reciprocal_approx_fast(rec_bc[:], psm[:])
                        nc.vector.tensor_tensor(
                            attn_outT[:, h, qsl], pav[:], rec_bc[:], Alu.mult)
                    # o_proj for this q-chunk
                    for mt in range(H // P):
                        msl = slice(mt * P, (mt + 1) * P)
                        ps = pso.tile([P, SC], f32, tag="opsum")
                        for ct in range(NHG):
                            nc.tensor.matmul(
                                ps[:], wo_t[:, ct, msl],
                                attn_outT[:, ct, qsl],
                                start=(ct == 0), stop=(ct == NHG - 1))
                        ot = ot_pool.tile([P, SC], f16, tag="ot")
                        nc.vector.tensor_copy(ot[:], ps[:])
                        nc.sync.dma_start(outT[msl, qsl], ot[:])

    nc.compile()
    return nc


# ---------------------------------------------------------------------------
# Host side: shard inputs, run SPMD, gather.
# ---------------------------------------------------------------------------

def _rope_cos_sin(seq_len, dim, base=10000.0):
    inv_freq = 1.0 / (base ** (np.arange(0, dim, 2, dtype=np.float32) / dim))
    t = np.arange(seq_len, dtype=np.float32)
    freqs = np.outer(t, inv_freq).astype(np.float32)
    emb = np.concatenate([freqs, freqs], -1)
    return np.cos(emb).astype(np.float32), np.sin(emb).astype(np.float32)


def make_in_maps(hidden_states, Wq_down, bq_down, Wkv_down, bkv_down,
                 Wq_up, bq_up, Wk_up, bk_up, Wv_up, bv_up, Wo, bo):
    cos, sin = _rope_cos_sin(S, ROPE_DIM)
    WqdT = np.ascontiguousarray(Wq_down.T).astype(F16)
    WkvdT = np.ascontiguousarray(Wkv_down.T).astype(F16)
    hsT = [np.ascontiguousarray(hidden_states[b].T).astype(F16)
           for b in range(B)]
    in_maps = []
    for c in range(8):
        b, j = c // 4, c % 4
        heads = [j, 4 + j, 8 + j, 12 + j]
        x1 = slice(j * P, (j + 1) * P)
        x2 = slice(512 + j * P, 512 + (j + 1) * P)
        vrows = np.concatenate(
            [np.arange(h * P, (h + 1) * P) for h in heads])
        in_maps.append(dict(
            hsT=hsT[b],
            WqdT=WqdT, WkvdT=WkvdT,
            bqd=np.ascontiguousarray(bq_down),
            bkvd=np.ascontiguousarray(bkv_down),
            WquT=np.ascontiguousarray(
                np.concatenate([Wq_up[x1], Wq_up[x2]], 0).T),
            WkuT=np.ascontiguousarray(
                np.concatenate([Wk_up[x1], Wk_up[x2]], 0).T),
            bqku=np.stack(
                [bq_up[x1], bq_up[x2], bk_up[x1], bk_up[x2]], axis=1).copy(),
            WvuT=np.ascontiguousarray(Wv_up[vrows].T),
            bvu=np.ascontiguousarray(bv_up[vrows][None, :]),
            WoT=np.ascontiguousarray(Wo[:, vrows].T).astype(F16),
            cosT=np.ascontiguousarray(cos[:, x1].T).astype(F16),
            sinT=np.ascontiguousarray(sin[:, x1].T).astype(F16),
            ones=np.ones((P, P), np.float16),
        ))
    return in_maps


_NC_CACHE = {}


def _get_nc():
    if "nc" not in _NC_CACHE:
        _NC_CACHE["nc"] = build_mla()
    return _NC_CACHE["nc"]


LAST_RESULTS = None  # BassKernelResults of the most recent kernel() call


def kernel(**inputs):
    global LAST_RESULTS
    nc = _get_nc()
    in_maps = make_in_maps(**inputs)
    trace = bool(int(os.environ.get("MLA_TRACE", "0")))
    kwargs = {}
    if trace:
        tc_env = os.environ.get("MLA_TRACE_CORES", "0,1,2,3,4,5,6,7")
        kwargs["trace_cores"] = [int(x) for x in tc_env.split(",")]
    res = run_bass_kernel_spmd(
        nc, in_maps, core_ids=list(range(8)), trace=trace, **kwargs)
    LAST_RESULTS = res
    bo = inputs["bo"]
    out = np.zeros((B, S, H), np.float32)
    for b in range(B):
        acc = res.results[b * 4]["outT"].astype(np.float32)
        for j in range(1, 4):
            acc = acc + res.results[b * 4 + j]["outT"]
        out[b] = acc.T + bo[None, :]
    return out
